# revision 8
# baseline (speedup 1.0000x reference)
"""LIF cell recurrence kernel for Trainium2 (Bass/Tile), 8-core SPMD.

Problem: I_in [T=128, N=262144] f32. Per node n (independent), over time t:
    v = BETA*v + I[t] - GAMMA*s ; s = (v > TAU) ; v = v * (1 - s)
Outputs (spikes, v_mem, spikes), each [T, N].

Strategy (data parallel over nodes, 32768 nodes/core = [128p x 256f]):
  Carry the pre-reset potential u_t. The whole step is ONE custom DVE op:
      u_t = min(BETA*u, relu((TAU_EPS - u)*K) - GAMMA) + I_t
  With K=1e9 and TAU_EPS = nextafter(TAU): for u <= TAU the relu term is
  >= K*ulp - GAMMA ~ 119 > BETA*u, so u_t = fl(BETA*u) + I (bit-exact
  non-spiked path); for u >= TAU_EPS the relu is exactly 0, so
  u_t = fl(-GAMMA + I) (bit-exact spiked path). No f32 value lies
  strictly between TAU and TAU_EPS, so the threshold decision is exact.
  Verified bit-identical to the jax reference chain in numpy.

  Per 16-step block, ScalarE converts the u block to bf16 d = u - TAU
  (one activation op), halving output DMA bytes. Host: spikes = (d > 0)
  (exact: bf16 rounding preserves the sign of d), v_mem = (d+1)*(1-s)
  (~2e-3 rel err from bf16, well under the 2e-2 gate).

  Engine roles per block: DVE runs 16 chained custom ops (state stays in
  SBUF, slice j-1 -> slice j of the block tile); Act converts to bf16;
  GpSimd triggers output DMA; SP triggers input DMA. Tiny toucher ops
  absorb DMA/WAR waits so no instruction carries more than one sync wait.
"""

import numpy as np

T = 128
N = 262144
NCORES = 8
NPC = N // NCORES          # 32768 nodes per core
P = 128                    # SBUF partitions
F = NPC // P               # 256 free-dim elements per partition
BETA = 0.95
GAMMA = 0.95
TAU = 1.0
TAU_EPS = float(np.nextafter(np.float32(TAU), np.float32(2.0)))
KSLOPE = 1e9
BLK = 16                   # time steps per DMA block
NBLK = T // BLK

_NC_CACHE = {}


def _register_lif_op():
    """Register the fused LIF-step custom DVE op in the process-local
    registry (the documented way to add one: append to dve_ops.OPS)."""
    import concourse.dve_ops as dve_ops
    from concourse.dve_spec import (
        C0, C1, C2, Spec, Src0, Src1, _has_src1, lower as dve_lower, minn, relu,
    )
    from concourse.dve_uop import DveOpSpec

    name = "LIF_STEP_ANT"
    for op in dve_ops.OPS:
        if op.name == name:
            return op
    spec = Spec(
        body=minn(Src0 * C0, relu((C1 - Src0) * C2) - C0) + Src1,
        reference=lambda in0, in1, s0, s1, imm2: (
            np.minimum(
                in0.astype(np.float32) * np.float32(s0),
                np.maximum(
                    (np.float32(s1) - in0.astype(np.float32)) * np.float32(imm2),
                    np.float32(0),
                ) - np.float32(s0),
            ) + in1
        ).astype(np.float32),
    )
    row = dve_ops._CUSTOM_DVE_ROW_BASE + len(dve_ops.OPS)
    assert row < 0x20, "custom DVE opcode rows exhausted"
    dve_ops._SUB_OPCODE_FOR_NAME[name] = row
    shas = {}
    for ver in ("v3", "v4"):
        d = DveOpSpec(
            name=name, opcode=row, uops=dve_lower(spec, ver=ver),
            rd1_en=_has_src1(spec),
        )
        shas[ver] = d.sha(ver)
    op = dve_ops.DveOp(name, spec, subdim=False, uops_sha=shas)
    dve_ops.OPS.append(op)
    dve_ops.CUSTOM_DVE_SPECS[name] = spec
    return op


def build_nc(t_steps=T, p=P, f=F, blk=BLK):
    import concourse.bass as bass
    import concourse.tile as tile
    from concourse import bacc, mybir

    lif_op = _register_lif_op()

    f32 = mybir.dt.float32
    bf16 = mybir.dt.bfloat16
    nblk = t_steps // blk

    nc = bacc.Bacc(
        "TRN2", target_bir_lowering=False, debug=False, num_devices=NCORES
    )
    x_in = nc.declare_dram_parameter("x", [t_steps, p, f], f32, isOutput=False)
    d_out = nc.declare_dram_parameter("d", [t_steps, p, f], bf16, isOutput=True)

    x_r = x_in[:].rearrange("t p f -> p t f")
    d_r = d_out[:].rearrange("t p f -> p t f")

    with tile.TileContext(nc) as tc:
        with (
            tc.tile_pool(name="xin", bufs=nblk) as xpool,
            tc.tile_pool(name="ub", bufs=2) as ubpool,
            tc.tile_pool(name="dout", bufs=2) as dpool,
            tc.tile_pool(name="state", bufs=1) as spool,
        ):
            zero = spool.tile([p, f], f32)
            nc.vector.memset(zero[:], 0.0)
            sink = spool.tile([p, 1], f32)

            # Input DMA plan: block 0 (in 4 chunks, for the earliest
            # possible first compute) and block 1 go through the Sync
            # queue, which is ready at t=0 (the Pool DGE preamble takes
            # ~7us). Later blocks are prefetched 3 ahead from GpSimd
            # (SWDGE gen is ~3.5x cheaper per descriptor than Sync's
            # HWDGE). Keeping only ~3 transfers in flight matters: DMA
            # queues round-robin, so a deeper prefetch starves the
            # earliest-needed block.
            xts = [
                xpool.tile([p, blk * f], f32, tag="xin", name=f"xt{b}")
                for b in range(nblk)
            ]
            c0 = blk // 4

            def issue_in_dma(b, engine):
                engine.dma_start(
                    xts[b][:].rearrange("p (b f) -> p b f", b=blk),
                    x_r[:, bass.ts(b, blk), :],
                )

            for cch in range(4):
                nc.sync.dma_start(
                    xts[0][:, bass.ts(cch, c0 * f)].rearrange(
                        "p (b f) -> p b f", b=c0
                    ),
                    x_r[:, bass.ts(cch, c0), :],
                )
            issue_in_dma(1, nc.sync)

            hblk = blk // 2
            prev = zero[:]        # u_{t-1}; zeros => step 0 gives u_0 = I_0
            for b in range(nblk):
                xt = xts[b]
                if b + 2 < nblk:
                    issue_in_dma(b + 2, nc.gpsimd)
                ub = ubpool.tile([p, blk * f], f32, tag="ub")
                # DVE touchers: absorb the DMA-in wait and the ub-recycle
                # (WAR vs Act's read two blocks ago) in separate tiny ops
                # so each instruction carries at most one sync wait.
                nc.vector.memset(ub[:, 0:1], 0.0)
                dt = dpool.tile([p, blk * f], bf16, tag="dout")
                # Output flush segments: halves normally; quarters for the
                # last block to shrink the end-of-kernel tail.
                segs = (
                    [(q * c0, c0) for q in range(4)]
                    if b == nblk - 1 else [(0, hblk), (hblk, hblk)]
                )
                seg_i = 0
                for j in range(blk):
                    if b == 0 and j % c0 == 0:
                        # wait for chunk j//c0 of the split first block
                        nc.vector.tensor_copy(sink[:], xt[:, j * f:j * f + 1])
                    elif j == 0:
                        nc.vector.tensor_copy(sink[:], xt[:, 0:1])
                    cur = ub[:, bass.ts(j, f)]
                    nc.vector._custom_dve(
                        lif_op, out=cur, in0=prev, in1=xt[:, bass.ts(j, f)],
                        s0=BETA, s1=TAU_EPS, imm2=KSLOPE,
                    )
                    prev = cur
                    s0_, sn_ = segs[seg_i]
                    if j == s0_ + sn_ - 1:
                        # Segment done: Act converts it to bf16 d while the
                        # DVE runs on; the Sync-queue DMA carries the single
                        # Act-done wait. Act toucher absorbs the d-tile WAR
                        # (out-DMA of the recycled slot) once per block.
                        if seg_i == 0:
                            nc.scalar.memzero(dt[:, 0:2])
                        nc.scalar.activation(
                            dt[:, s0_ * f:(s0_ + sn_) * f],
                            ub[:, s0_ * f:(s0_ + sn_) * f],
                            mybir.ActivationFunctionType.Copy,
                            bias=-TAU, scale=1.0,
                        )
                        nc.sync.dma_start(
                            d_r[:, b * blk + s0_:b * blk + s0_ + sn_, :],
                            dt[:, s0_ * f:(s0_ + sn_) * f].rearrange(
                                "p (b f) -> p b f", b=sn_
                            ),
                        )
                        seg_i += 1
    nc.compile()
    return nc


def _get_nc():
    if "nc" not in _NC_CACHE:
        _NC_CACHE["nc"] = build_nc()
    return _NC_CACHE["nc"]


def run_device(I_in, trace=False, trace_kwargs=None):
    """Run the Bass kernel on 8 cores; return (d_full [T,N] f32, results)."""
    from concourse.bass_utils import run_bass_kernel_spmd

    nc = _get_nc()
    I_in = np.ascontiguousarray(I_in, dtype=np.float32)
    in_maps = [
        {"x": I_in[:, c * NPC:(c + 1) * NPC].reshape(T, P, F)}
        for c in range(NCORES)
    ]
    kw = {}
    if trace:
        kw["trace"] = True
        if trace_kwargs:
            kw["trace_kwargs"] = trace_kwargs
    res = run_bass_kernel_spmd(nc, in_maps, list(range(NCORES)), **kw)
    d_full = np.empty((T, N), dtype=np.float32)
    for c in range(NCORES):
        d_full[:, c * NPC:(c + 1) * NPC] = np.asarray(
            res.results[c]["d"]
        ).astype(np.float32).reshape(T, NPC)
    return d_full, res


def kernel(I_in):
    d_full, _ = run_device(I_in)
    spikes = (d_full > np.float32(0.0)).astype(np.float32)
    v_mem = (d_full + np.float32(TAU)) * (np.float32(1.0) - spikes)
    return spikes, v_mem, spikes
